# revision 50
# baseline (speedup 1.0000x reference)
"""EdgeConv GNN (4 layers) on 8 Trainium2 NeuronCores.

Algebraic restructure: with y = x @ theta_w.T and
v = x @ (phi_w - theta_w).T + (phi_b + theta_b),
    msg_e = theta(x[src]-x[dst]) + theta_b + phi(x[dst]) + phi_b
          = y[src] + v[dst]
and since v[dst] is constant within a dst segment:
    out = relu(v + segment_max(y[src], dst))
(nodes with no in-edges come out of segment_max very negative -> relu -> 0,
matching the reference's where(isneginf, 0) + relu).

Distribution: nodes sharded by dst across 8 cores (graph parallel).
The per-core node order (pos) is split in two halves: pos < 3200 ("half 1",
blocks 0-24) and pos >= 3200 ("half 2", blocks 25-48).  Each layer the y
table (fp16) is exchanged with TWO AllGathers - one per half - so AG1 can
fire as soon as blocks 0-24 are computed (mid reduce-phase of the previous
layer) and AG2 overlaps with the next layer's window-A gathers.

Window A gathers from table1 (8*3200 = 25600 rows), window B from table2
(8*3072 = 24576 rows); both fit int16 indices directly.  Every src node is
assigned to one half (greedy, balancing each dst's in-degree split), so each
edge belongs to exactly one window.  Per-core slot structure must be
identical across cores (single SPMD instruction stream), so block degree
caps K are maxima across all 8 cores.

The critical resource is the Pool engine: SWDGE descriptor generation for
dma_gather runs at ~8ns/index, so everything else (matmuls, reduces, both
AllGathers, the DMA drain itself) is overlapped under the gather stream:
 - window-A gathers of each layer run while that layer's AG2 is in flight
   (A only needs AG1, which fired mid-previous-layer);
 - the next layer's matmuls are emitted per block inside the reduce phase
   (transpose one block ahead of the matmul to keep the PE queue moving);
 - layer 0's lhsT comes pre-transposed fp16 from the host;
 - gB tiles are allocated + NEG16-memset one group ahead so the memset
   (which backs the trailing-index trim) never stalls the Pool queue;
 - each group's common trailing dummy run is -1 and skipped via
   num_idxs_reg (lanes within a block are ordered by descending degree so
   dummies cluster at the tail).
"""

import numpy as np

N = 50000
NCORES = 8
NPC = 6250            # real nodes per core
NPCP = 6272           # padded nodes per core (49 * 128)
F = 128
NL = 4
NB = NPCP // 128      # 49 blocks per core
H1 = 3200             # pos in half 1 (blocks 0-24)
H2 = 3072             # pos in half 2 (blocks 25-48)
NB1 = H1 // 128       # 25
NPH = 11              # phantom rows per half (block 0 / block 25, lanes 0-10)
R1 = H1 - NPH         # real nodes in half 1
R2 = H2 - NPH
T1 = NCORES * H1      # table1 rows (25600)
T2 = NCORES * H2      # table2 rows (24576)
GMAX = 20             # max chunks per gather tile (per window)
GCALL = 8             # chunks per dma_gather call (ucode caps num_idxs at 1024)
LA = 4                # A-gather group lookahead (hides AG2 latency)
NEG = -1.0e30
NEG16 = -60000.0      # phantom-row marker in the fp16 y table

_cache = {}


# ----------------------------------------------------------------------------
# host-side graph preprocessing
# ----------------------------------------------------------------------------

def _prep_graph(src, dst):
    src = np.asarray(src).astype(np.int64)
    dst = np.asarray(dst).astype(np.int64)

    # adjacency by src
    order_e = np.argsort(src, kind="stable")
    s_sorted = src[order_e]
    dstlist = dst[order_e]
    starts = np.searchsorted(s_sorted, np.arange(N))
    ends = np.searchsorted(s_sorted, np.arange(N) + 1)
    outdeg = ends - starts

    # greedy half assignment per core: process srcs by decreasing out-degree,
    # put each src in the half that balances its dsts' in-window degrees
    halfA = np.zeros(N, bool)
    dA = np.zeros(N, np.int32)
    dB = np.zeros(N, np.int32)
    for c in range(NCORES):
        ids = np.arange(c * NPC, (c + 1) * NPC)
        proc = ids[np.argsort(-outdeg[ids], kind="stable")]
        capA, capB = R1, R2
        for j in proc:
            Dj = dstlist[starts[j]:ends[j]]
            toA = ((dA[Dj] - dB[Dj]).sum() < 0 or capB == 0) and capA > 0
            if toA:
                halfA[j] = True
                dA[Dj] += 1
                capA -= 1
            else:
                dB[Dj] += 1
                capB -= 1

    # per-core node order: within each half, nodes sorted ascending by
    # (max(dA,dB), min(dA,dB)) for tight per-block caps, then lane-REVERSED
    # within each 128-block so high-degree lanes come first (phantoms and the
    # sparsest lanes land at the block tail, enabling trailing-index trim).
    kmax = np.maximum(dA, dB).astype(np.int64)
    kmin = np.minimum(dA, dB).astype(np.int64)

    def rev(p):
        return (p // 128) * 128 + (127 - (p % 128))

    pos = np.empty(N, np.int64)
    for c in range(NCORES):
        ids = np.arange(c * NPC, (c + 1) * NPC)
        a_ids = ids[halfA[ids]]
        b_ids = ids[~halfA[ids]]
        assert len(a_ids) == R1 and len(b_ids) == R2
        oa = a_ids[np.lexsort((kmin[a_ids], kmax[a_ids]))]
        ob = b_ids[np.lexsort((kmin[b_ids], kmax[b_ids]))]
        pos[oa] = rev(NPH + np.arange(R1))
        pos[ob] = H1 + rev(NPH + np.arange(R2))
    core = np.arange(N) // NPC
    # table rows: half1 -> table1 row c*H1+pos; half2 -> table2 row c*H2+pos-H1
    sig = np.where(halfA, core * H1 + pos, core * H2 + (pos - H1))
    blk = pos // 128
    lane = pos % 128

    # global (cross-core) block degree caps
    KA = np.zeros(NB, np.int64)
    KB = np.zeros(NB, np.int64)
    np.maximum.at(KA, blk, dA)
    np.maximum.at(KB, blk, dB)
    cbA = np.r_[0, np.cumsum(KA)]
    cbB = np.r_[0, np.cumsum(KB)]
    CA, CB = int(cbA[-1]), int(cbB[-1])
    assert KA.max() <= GMAX and KB.max() <= GMAX, (KA.max(), KB.max())

    # slot arrays (per core); interior dummies point at phantom row 127 of
    # each table (= core 0's block-0 lane-127 phantom -> NEG16).  The common
    # (all-cores) trailing dummy run of each block is set to -1 and skipped by
    # the gather via num_idxs_reg (trim*); the uncovered lanes of the last
    # chunk are NEG16-memset on chip.
    idxA = np.full((NCORES, CA * 128), 127, np.int16)
    idxB = np.full((NCORES, CB * 128), 127, np.int16)
    d_core = dst // NPC

    for win, idx_arr, cb, K in ((True, idxA, cbA, KA),
                                (False, idxB, cbB, KB)):
        e = np.flatnonzero(halfA[src] == win)
        e = e[np.argsort(dst[e], kind="stable")]
        de = dst[e]
        st = np.r_[0, np.flatnonzero(np.diff(de)) + 1]
        runlen = np.diff(np.r_[st, len(de)])
        rank = np.arange(len(de)) - np.repeat(st, runlen)
        slot = (cb[blk[de]] + rank) * 128 + lane[de]
        val = sig[src[e]]
        assert val.min() >= 0 and val.max() < 32768, (val.min(), val.max())
        idx_arr[d_core[e], slot] = val.astype(np.int16)

    # wrap indices: [n] -> [128, n//16] int16, replicated across 8 groups of 16
    def wrap(a):
        n = a.shape[1]
        w = a.reshape(NCORES, n // 16, 16).transpose(0, 2, 1)
        return np.ascontiguousarray(
            np.broadcast_to(w[:, None, :, :], (NCORES, 8, 16, n // 16))
        ).reshape(NCORES, 128, n // 16)

    # gather groups: consecutive blocks, chunk budget GMAX per window
    groups = []
    b0 = 0
    while b0 < NB:
        nb = 1
        while (
            b0 + nb < NB
            and cbA[b0 + nb + 1] - cbA[b0] <= GMAX
            and cbB[b0 + nb + 1] - cbB[b0] <= GMAX
        ):
            nb += 1
        groups.append((b0, nb, int(cbA[b0]), int(cbA[b0 + nb] - cbA[b0]),
                       int(cbB[b0]), int(cbB[b0 + nb] - cbB[b0])))
        b0 += nb

    # per-(window, group) trailing dummy run -> -1 (skipped by the gather via
    # num_idxs_reg); only the run at the END of a group's chunk range can be
    # trimmed since -1s must be trailing within each dma_gather call.
    trimA = []
    trimB = []
    for (b0, nbl, aoff, acnt, boff, bcnt) in groups:
        for idx_arr, off, cnt, trim in ((idxA, aoff, acnt, trimA),
                                        (idxB, boff, bcnt, trimB)):
            if cnt == 0:
                trim.append(0)
                continue
            s0, s1 = off * 128, (off + cnt) * 128
            occ = (idx_arr[:, s0:s1] != 127).any(0)  # real on ANY core
            last = int(np.flatnonzero(occ).max()) if occ.any() else 0
            lastcall = ((cnt - 1) // GCALL) * GCALL * 128  # last call's base
            L = max(last + 1, lastcall + min(256, s1 - s0 - lastcall))
            trim.append(L)
            if s0 + L < s1:
                idx_arr[:, s0 + L : s1] = -1

    return dict(
        sig=sig, pos=pos, halfA=halfA, KA=KA, KB=KB, cbA=cbA, cbB=cbB,
        CA=CA, CB=CB, idxA=wrap(idxA), idxB=wrap(idxB), groups=groups,
        idxA_flat=idxA, idxB_flat=idxB, trimA=trimA, trimB=trimB,
    )


def _prep_weights(theta_w, theta_b, phi_w, phi_b):
    theta_w = np.asarray(theta_w, np.float32)
    phi_w = np.asarray(phi_w, np.float32)
    cb = (np.asarray(theta_b, np.float32) + np.asarray(phi_b, np.float32))
    wcat = np.concatenate(
        [theta_w.transpose(0, 2, 1), (phi_w - theta_w).transpose(0, 2, 1)], axis=2
    )  # [NL, 128(in), 256(out: y|v)]
    return np.ascontiguousarray(wcat.astype(np.float16)), np.ascontiguousarray(cb)


# ----------------------------------------------------------------------------
# device kernel
# ----------------------------------------------------------------------------

def _build_kernel(g):
    import concourse.bacc as bacc
    import concourse.mybir as mybir
    import concourse.tile as tile
    from concourse.masks import make_identity

    KA, KB, groups = g["KA"], g["KB"], g["groups"]
    cbA, cbB = g["cbA"], g["cbB"]
    CA, CB = g["CA"], g["CB"]
    trimA, trimB = g["trimA"], g["trimB"]
    ngroups = len(groups)

    nc = bacc.Bacc("TRN2", target_bir_lowering=False, debug=False,
                   num_devices=NCORES, num_swdge_queues=4)

    xin = nc.dram_tensor("xin", [128, NB * 128], mybir.dt.float16, kind="ExternalInput")
    idxA_in = nc.dram_tensor("idxA", [128, CA * 8], mybir.dt.int16, kind="ExternalInput")
    idxB_in = nc.dram_tensor("idxB", [128, CB * 8], mybir.dt.int16, kind="ExternalInput")
    wcat_in = nc.dram_tensor("wcat", [NL, F, 2 * F], mybir.dt.float16, kind="ExternalInput")
    cb_in = nc.dram_tensor("cb", [NL, F], mybir.dt.float32, kind="ExternalInput")
    xout = nc.dram_tensor("xout", [NPCP, F], mybir.dt.float32, kind="ExternalOutput")

    fp32 = mybir.dt.float32
    fp16 = mybir.dt.float16
    Alu = mybir.AluOpType
    Act = mybir.ActivationFunctionType

    with tile.TileContext(nc) as tc:
        with (
            tc.tile_pool(name="const", bufs=1) as constp,
            tc.tile_pool(name="xp", bufs=2) as xp,
            tc.tile_pool(name="vp", bufs=2) as vp,
            tc.tile_pool(name="wp", bufs=2) as wp,
            tc.tile_pool(name="yp", bufs=3) as yp,
            tc.tile_pool(name="xtp", bufs=3) as xtp,
            tc.tile_pool(name="ga", bufs=LA + 2) as gap,
            tc.tile_pool(name="gb", bufs=6) as gbp,
            tc.tile_pool(name="tp", bufs=4) as tp,
            tc.tile_pool(name="ps", bufs=4, space="PSUM") as ps,
            tc.tile_pool(name="dram", bufs=2, space="DRAM") as dram,
        ):
            ident = constp.tile([128, 128], fp32)
            make_identity(nc, ident[:])
            neg_ph = constp.tile([NPH, F], fp16)
            nc.vector.memset(neg_ph[:], NEG16)
            idxA = constp.tile([128, CA * 8], mybir.dt.int16)
            idxB = constp.tile([128, CB * 8], mybir.dt.int16)
            nc.sync.dma_start(idxA[:], idxA_in[:])
            nc.sync.dma_start(idxB[:], idxB_in[:])

            # layer-0 lhsT comes pre-transposed (and fp16) from the host
            xT0 = constp.tile([128, NB, 128], fp16)
            nc.sync.dma_start(xT0[:], xin.rearrange("p (b l) -> p b l", b=NB))

            # per-layer weight/collective state
            def load_weights(l):
                W = wp.tile([128, 2 * F], fp16, tag="w")
                nc.sync.dma_start(W[:], wcat_in[l])
                cb_sb = wp.tile([1, F], fp32, tag="cb")
                nc.sync.dma_start(cb_sb[:], cb_in[l : l + 1, :])
                cbbc = wp.tile([128, F], fp32, tag="cbbc")
                nc.gpsimd.partition_broadcast(cbbc[:], cb_sb[:])
                return W, cbbc

            def new_tables():
                y1 = dram.tile([H1, F], fp16, tag="y1")
                y2 = dram.tile([H2, F], fp16, tag="y2")
                ya1 = dram.tile([T1, F], fp16, tag="ya1", addr_space="Shared")
                ya2 = dram.tile([T2, F], fp16, tag="ya2", addr_space="Shared")
                return y1, y2, ya1, ya2

            def ag(y, ya):
                nc.gpsimd.collective_compute(
                    "AllGather", Alu.bypass,
                    replica_groups=[list(range(NCORES))],
                    ins=[y.opt()], outs=[ya.opt()],
                )

            def mm_block(b, lhsT, W, cbbc, v, y1, y2):
                yv_ps = ps.tile([128, 2 * F], fp32, tag="yv_ps")
                nc.tensor.matmul(yv_ps[:], lhsT=lhsT, rhs=W[:],
                                 start=True, stop=True)
                y_sb = yp.tile([128, F], fp16, tag="y")
                nc.scalar.activation(y_sb[:], yv_ps[:, 0:F], Act.Copy)
                yt = y1 if b < NB1 else y2
                r0 = (b if b < NB1 else b - NB1) * 128
                if b == 0 or b == NB1:
                    # lanes 117-127 are phantom rows: engines can't address a
                    # partition slice at 117, so write them from a const tile
                    nc.sync.dma_start(yt[r0 : r0 + 128 - NPH, :],
                                      y_sb[0 : 128 - NPH, :])
                    nc.sync.dma_start(yt[r0 + 128 - NPH : r0 + 128, :], neg_ph[:])
                else:
                    nc.sync.dma_start(yt[r0 : r0 + 128, :], y_sb[:])
                nc.vector.tensor_tensor(out=v[:, b, :], in0=yv_ps[:, F : 2 * F],
                                        in1=cbbc[:], op=Alu.add)

            def win_memset(g_tile, cnt, L):
                # skipped trailing lanes of the group's last chunk read as -inf
                if 0 < L < cnt * 128:
                    nc.vector.memset(g_tile[:, (L // 128) : cnt, :], NEG16)

            qn = [0]

            def win_calls(g_tile, ya, idx_sb, off, cnt, L):
                for o in range(0, cnt, GCALL):
                    n = min(GCALL, cnt - o)
                    reg = min(n * 128, L - o * 128)
                    qn[0] = (qn[0] + 1) % 4
                    nc.gpsimd.dma_gather(
                        g_tile[:, o : o + n, :], ya[:, :],
                        idx_sb[:, (off + o) * 8 : (off + o + n) * 8],
                        n * 128, reg, F,
                        queue_num=qn[0],
                    )

            def a_calls(gi, ya1):
                (b0, nbl, aoff, acnt, boff, bcnt) = groups[gi]
                gA = gap.tile([128, GMAX, F], fp16, tag="ga")
                win_memset(gA, acnt, int(trimA[gi]))
                win_calls(gA, ya1, idxA, aoff, acnt, int(trimA[gi]))
                return gA

            def b_tile(gi):
                # allocate + memset one group ahead so the DVE memset never
                # stalls the Pool engine's gather stream
                gB = gbp.tile([128, GMAX, F], fp16, tag="gb")
                win_memset(gB, groups[gi][5], int(trimB[gi]))
                return gB

            # ---- layer 0 matmuls (lhsT pre-transposed on host) ----
            W, cbbc = load_weights(0)
            v = vp.tile([128, NB, F], fp32, tag="v")
            y1, y2, ya1, ya2 = new_tables()
            for b in range(NB):
                mm_block(b, xT0[:, b, :], W, cbbc, v, y1, y2)
                if b == NB1 - 1:
                    ag(y1, ya1)

            for l in range(NL):
                last = l == NL - 1
                if not last:
                    Wn, cbbcn = load_weights(l + 1)
                    vn = vp.tile([128, NB, F], fp32, tag="v")
                    y1n, y2n, ya1n, ya2n = new_tables()
                x_next = xp.tile([128, NB, F], fp32, tag="x")

                # AG2 sits on the Pool queue and holds its SEQ while waiting
                # for the y2 writes; emitting it after one A-call group lets
                # those gathers (which only need AG1) start first, without
                # delaying the AG2 trigger behind too much descriptor-gen.
                ga_tiles = {0: a_calls(0, ya1)}
                ag(y2, ya2)
                for gi in range(1, min(LA, ngroups)):
                    ga_tiles[gi] = a_calls(gi, ya1)

                gb_tiles = {0: b_tile(0)}
                pend = []  # software-pipelined (b, xT) awaiting matmul

                def flush_mm(limit):
                    while len(pend) > limit:
                        bb, xt = pend.pop(0)
                        mm_block(bb, xt[:], Wn, cbbcn, vn, y1n, y2n)
                        if bb == NB1 - 1:
                            ag(y1n, ya1n)

                for gi, (b0, nbl, aoff, acnt, boff, bcnt) in enumerate(groups):
                    gB = gb_tiles.pop(gi)
                    win_calls(gB, ya2, idxB, boff, bcnt, int(trimB[gi]))
                    if gi + 1 < ngroups:
                        gb_tiles[gi + 1] = b_tile(gi + 1)
                    if gi + LA < ngroups:
                        ga_tiles[gi + LA] = a_calls(gi + LA, ya1)
                    gA = ga_tiles.pop(gi)

                    ka = cbA[b0] - aoff
                    kb = cbB[b0] - boff
                    for b in range(b0, b0 + nbl):
                        ha, hb = int(KA[b]), int(KB[b])
                        tS = tp.tile([128, F], fp32, tag="ts")
                        if ha > 0 and hb > 0:
                            tA = tp.tile([128, F], fp32, tag="ta")
                            tB = tp.tile([128, F], fp32, tag="tb")
                            nc.vector.tensor_reduce(
                                out=tA[:],
                                in_=gA[:, ka : ka + ha, :].rearrange("p c f -> p f c"),
                                axis=mybir.AxisListType.X, op=Alu.max)
                            nc.vector.tensor_reduce(
                                out=tB[:],
                                in_=gB[:, kb : kb + hb, :].rearrange("p c f -> p f c"),
                                axis=mybir.AxisListType.X, op=Alu.max)
                            tM = tp.tile([128, F], fp32, tag="tm")
                            nc.vector.tensor_tensor(out=tM[:], in0=tA[:], in1=tB[:],
                                                    op=Alu.max)
                            nc.vector.tensor_tensor(out=tS[:], in0=tM[:],
                                                    in1=v[:, b, :], op=Alu.add)
                        elif ha > 0 or hb > 0:
                            tA = tp.tile([128, F], fp32, tag="ta")
                            src_g = (gA, ka, ha) if ha > 0 else (gB, kb, hb)
                            nc.vector.tensor_reduce(
                                out=tA[:],
                                in_=src_g[0][:, src_g[1] : src_g[1] + src_g[2], :]
                                    .rearrange("p c f -> p f c"),
                                axis=mybir.AxisListType.X, op=Alu.max)
                            nc.vector.tensor_tensor(out=tS[:], in0=tA[:],
                                                    in1=v[:, b, :], op=Alu.add)
                        else:
                            nc.vector.memset(tS[:], NEG)
                        nc.scalar.activation(x_next[:, b, :], tS[:], Act.Relu)
                        if not last:
                            # transpose now, matmul one block behind - keeps
                            # the PE queue from stalling on the ACT xT copy
                            xT_ps = ps.tile([128, 128], fp32, tag="xt_ps")
                            nc.tensor.transpose(xT_ps[:], x_next[:, b, :], ident[:])
                            xT = xtp.tile([128, 128], fp16, tag="xt")
                            nc.scalar.activation(xT[:], xT_ps[:], Act.Copy)
                            pend.append((b, xT))
                            flush_mm(1)
                        ka += ha
                        kb += hb
                if not last:
                    flush_mm(0)
                    W, cbbc, v = Wn, cbbcn, vn
                    y1, y2, ya1, ya2 = y1n, y2n, ya1n, ya2n
                x = x_next

            nc.sync.dma_start(xout.rearrange("(b p) f -> p b f", p=128), x[:])

    nc.compile()
    return nc


# ----------------------------------------------------------------------------
# numpy emulation of the device dataflow (for validating prep structures)
# ----------------------------------------------------------------------------

def _emulate(g, feats_dev, wcat, cb):
    KA, KB = g["KA"], g["KB"]
    x = feats_dev.copy()  # [NCORES, NPCP, F] pos-ordered
    for l in range(NL):
        w = wcat[l].astype(np.float32)
        x16 = x.astype(np.float16).astype(np.float32)
        y_sh = np.einsum("cnf,fk->cnk", x16, w[:, :F]).astype(np.float16)
        v = np.einsum("cnf,fk->cnk", x16, w[:, F:]) + cb[l]
        y_sh[:, 128 - NPH : 128, :] = NEG16
        y_sh[:, H1 + 128 - NPH : H1 + 128, :] = NEG16
        t1 = np.ascontiguousarray(y_sh[:, :H1, :]).reshape(T1, F).astype(np.float32)
        t2 = np.ascontiguousarray(y_sh[:, H1:, :]).reshape(T2, F).astype(np.float32)
        xn = np.empty_like(x)
        for c in range(NCORES):
            iA = g["idxA_flat"][c].astype(np.int64)
            iB = g["idxB_flat"][c].astype(np.int64)
            gA = np.where((iA >= 0)[:, None], t1[np.maximum(iA, 0)], NEG16)
            gB = np.where((iB >= 0)[:, None], t2[np.maximum(iB, 0)], NEG16)
            gA = gA.reshape(g["CA"], 128, F)
            gB = gB.reshape(g["CB"], 128, F)
            for b in range(NB):
                a0, b0 = g["cbA"][b], g["cbB"][b]
                parts = []
                if KA[b] > 0:
                    parts.append(gA[a0 : a0 + KA[b]].max(0))
                if KB[b] > 0:
                    parts.append(gB[b0 : b0 + KB[b]].max(0))
                agg = np.full((128, F), NEG, np.float32) if not parts else (
                    parts[0] if len(parts) == 1 else np.maximum(*parts))
                xn[c, b * 128 : (b + 1) * 128] = np.maximum(
                    agg + v[c, b * 128 : (b + 1) * 128], 0.0)
        x = xn
    return x


def _make_in_maps(g, feats_dev, wcat, cb):
    in_maps = []
    for c in range(NCORES):
        in_maps.append({
            "xin": np.ascontiguousarray(feats_dev[c].T.astype(np.float16)),
            "idxA": np.ascontiguousarray(g["idxA"][c]),
            "idxB": np.ascontiguousarray(g["idxB"][c]),
            "wcat": wcat,
            "cb": cb,
        })
    return in_maps


def _feats_dev(g, feats):
    feats = np.asarray(feats, np.float32)
    fd = np.zeros((NCORES, NPCP, F), np.float32)
    core = np.arange(N) // NPC
    fd[core, g["pos"]] = feats
    return fd


def _assemble(g, results):
    out_sh = np.stack([r["xout"] for r in results])  # [NCORES, NPCP, F]
    core = np.arange(N) // NPC
    return np.ascontiguousarray(out_sh[core, g["pos"]])


def run(feats, src, dst, theta_w, theta_b, phi_w, phi_b, trace=False):
    from concourse.bass_utils import run_bass_kernel_spmd

    key = (src.tobytes()[:64], dst.tobytes()[:64], len(src))
    if _cache.get("graph_key") != key:
        _cache.clear()
        _cache["graph"] = _prep_graph(src, dst)
        _cache["graph_key"] = key
    g = _cache["graph"]
    if "nc" not in _cache:
        _cache["nc"] = _build_kernel(g)
    nc = _cache["nc"]

    wcat, cb = _prep_weights(theta_w, theta_b, phi_w, phi_b)
    feats_dev = _feats_dev(g, feats)
    in_maps = _make_in_maps(g, feats_dev, wcat, cb)
    res = run_bass_kernel_spmd(nc, in_maps, core_ids=list(range(NCORES)),
                               trace=trace)
    out = _assemble(g, res.results)
    return out, res


def kernel(feats, src, dst, theta_w, theta_b, phi_w, phi_b):
    out, _ = run(feats, src, dst, theta_w, theta_b, phi_w, phi_b)
    return out


# revision 51
# speedup vs baseline: 1.0069x; 1.0069x over previous
"""EdgeConv GNN (4 layers) on 8 Trainium2 NeuronCores.

Algebraic restructure: with y = x @ theta_w.T and
v = x @ (phi_w - theta_w).T + (phi_b + theta_b),
    msg_e = theta(x[src]-x[dst]) + theta_b + phi(x[dst]) + phi_b
          = y[src] + v[dst]
and since v[dst] is constant within a dst segment:
    out = relu(v + segment_max(y[src], dst))
(nodes with no in-edges come out of segment_max very negative -> relu -> 0,
matching the reference's where(isneginf, 0) + relu).

Distribution: nodes sharded by dst across 8 cores (graph parallel).
The per-core node order (pos) is split in two halves: pos < 3200 ("half 1",
blocks 0-24) and pos >= 3200 ("half 2", blocks 25-48).  Each layer the y
table (fp16) is exchanged with TWO AllGathers - one per half - so AG1 can
fire as soon as blocks 0-24 are computed (mid reduce-phase of the previous
layer) and AG2 overlaps with the next layer's window-A gathers.

Window A gathers from table1 (8*3200 = 25600 rows), window B from table2
(8*3072 = 24576 rows); both fit int16 indices directly.  Every src node is
assigned to one half (greedy, balancing each dst's in-degree split), so each
edge belongs to exactly one window.  Per-core slot structure must be
identical across cores (single SPMD instruction stream), so block degree
caps K are maxima across all 8 cores.

The critical resource is the Pool engine: SWDGE descriptor generation for
dma_gather runs at ~8ns/index, so everything else (matmuls, reduces, both
AllGathers, the DMA drain itself) is overlapped under the gather stream:
 - window-A gathers of each layer run while that layer's AG2 is in flight
   (A only needs AG1, which fired mid-previous-layer);
 - the next layer's matmuls are emitted per block inside the reduce phase
   (transpose one block ahead of the matmul to keep the PE queue moving);
 - layer 0's lhsT comes pre-transposed fp16 from the host;
 - gB tiles are allocated + NEG16-memset one group ahead so the memset
   (which backs the trailing-index trim) never stalls the Pool queue;
 - each group's common trailing dummy run is -1 and skipped via
   num_idxs_reg (lanes within a block are ordered by descending degree so
   dummies cluster at the tail).
"""

import numpy as np

N = 50000
NCORES = 8
NPC = 6250            # real nodes per core
NPCP = 6272           # padded nodes per core (49 * 128)
F = 128
NL = 4
NB = NPCP // 128      # 49 blocks per core
H1 = 3200             # pos in half 1 (blocks 0-24)
H2 = 3072             # pos in half 2 (blocks 25-48)
NB1 = H1 // 128       # 25
NPH = 11              # phantom rows per half (block 0 / block 25, lanes 0-10)
R1 = H1 - NPH         # real nodes in half 1
R2 = H2 - NPH
T1 = NCORES * H1      # table1 rows (25600)
T2 = NCORES * H2      # table2 rows (24576)
GMAX = 20             # max chunks per gather tile (per window)
GCALL = 8             # chunks per dma_gather call (ucode caps num_idxs at 1024)
LA = 4                # A-gather group lookahead (hides AG2 latency)
NEG = -1.0e30
NEG16 = -60000.0      # phantom-row marker in the fp16 y table

_cache = {}


# ----------------------------------------------------------------------------
# host-side graph preprocessing
# ----------------------------------------------------------------------------

def _prep_graph(src, dst):
    src = np.asarray(src).astype(np.int64)
    dst = np.asarray(dst).astype(np.int64)

    # adjacency by src
    order_e = np.argsort(src, kind="stable")
    s_sorted = src[order_e]
    dstlist = dst[order_e]
    starts = np.searchsorted(s_sorted, np.arange(N))
    ends = np.searchsorted(s_sorted, np.arange(N) + 1)
    outdeg = ends - starts

    # greedy half assignment per core: process srcs by decreasing out-degree,
    # put each src in the half that balances its dsts' in-window degrees
    halfA = np.zeros(N, bool)
    dA = np.zeros(N, np.int32)
    dB = np.zeros(N, np.int32)
    for c in range(NCORES):
        ids = np.arange(c * NPC, (c + 1) * NPC)
        proc = ids[np.argsort(-outdeg[ids], kind="stable")]
        capA, capB = R1, R2
        for j in proc:
            Dj = dstlist[starts[j]:ends[j]]
            toA = ((dA[Dj] - dB[Dj]).sum() < 0 or capB == 0) and capA > 0
            if toA:
                halfA[j] = True
                dA[Dj] += 1
                capA -= 1
            else:
                dB[Dj] += 1
                capB -= 1

    # per-core node order: within each half, nodes sorted ascending by
    # (max(dA,dB), min(dA,dB)) for tight per-block caps, then lane-REVERSED
    # within each 128-block so high-degree lanes come first (phantoms and the
    # sparsest lanes land at the block tail, enabling trailing-index trim).
    kmax = np.maximum(dA, dB).astype(np.int64)
    kmin = np.minimum(dA, dB).astype(np.int64)

    def rev(p):
        return (p // 128) * 128 + (127 - (p % 128))

    pos = np.empty(N, np.int64)
    for c in range(NCORES):
        ids = np.arange(c * NPC, (c + 1) * NPC)
        a_ids = ids[halfA[ids]]
        b_ids = ids[~halfA[ids]]
        assert len(a_ids) == R1 and len(b_ids) == R2
        oa = a_ids[np.lexsort((kmin[a_ids], kmax[a_ids]))]
        ob = b_ids[np.lexsort((kmin[b_ids], kmax[b_ids]))]
        pos[oa] = rev(NPH + np.arange(R1))
        pos[ob] = H1 + rev(NPH + np.arange(R2))
    core = np.arange(N) // NPC
    # table rows: half1 -> table1 row c*H1+pos; half2 -> table2 row c*H2+pos-H1
    sig = np.where(halfA, core * H1 + pos, core * H2 + (pos - H1))
    blk = pos // 128
    lane = pos % 128

    # global (cross-core) block degree caps
    KA = np.zeros(NB, np.int64)
    KB = np.zeros(NB, np.int64)
    np.maximum.at(KA, blk, dA)
    np.maximum.at(KB, blk, dB)
    cbA = np.r_[0, np.cumsum(KA)]
    cbB = np.r_[0, np.cumsum(KB)]
    CA, CB = int(cbA[-1]), int(cbB[-1])
    assert KA.max() <= GMAX and KB.max() <= GMAX, (KA.max(), KB.max())

    # slot arrays (per core); interior dummies point at phantom row 127 of
    # each table (= core 0's block-0 lane-127 phantom -> NEG16).  The common
    # (all-cores) trailing dummy run of each block is set to -1 and skipped by
    # the gather via num_idxs_reg (trim*); the uncovered lanes of the last
    # chunk are NEG16-memset on chip.
    idxA = np.full((NCORES, CA * 128), 127, np.int16)
    idxB = np.full((NCORES, CB * 128), 127, np.int16)
    d_core = dst // NPC

    for win, idx_arr, cb, K in ((True, idxA, cbA, KA),
                                (False, idxB, cbB, KB)):
        e = np.flatnonzero(halfA[src] == win)
        e = e[np.argsort(dst[e], kind="stable")]
        de = dst[e]
        st = np.r_[0, np.flatnonzero(np.diff(de)) + 1]
        runlen = np.diff(np.r_[st, len(de)])
        rank = np.arange(len(de)) - np.repeat(st, runlen)
        slot = (cb[blk[de]] + rank) * 128 + lane[de]
        val = sig[src[e]]
        assert val.min() >= 0 and val.max() < 32768, (val.min(), val.max())
        idx_arr[d_core[e], slot] = val.astype(np.int16)

    # wrap indices: [n] -> [128, n//16] int16, replicated across 8 groups of 16
    def wrap(a):
        n = a.shape[1]
        w = a.reshape(NCORES, n // 16, 16).transpose(0, 2, 1)
        return np.ascontiguousarray(
            np.broadcast_to(w[:, None, :, :], (NCORES, 8, 16, n // 16))
        ).reshape(NCORES, 128, n // 16)

    # gather groups: consecutive blocks, chunk budget GMAX per window
    groups = []
    b0 = 0
    while b0 < NB:
        nb = 1
        while (
            b0 + nb < NB
            and cbA[b0 + nb + 1] - cbA[b0] <= GMAX
            and cbB[b0 + nb + 1] - cbB[b0] <= GMAX
        ):
            nb += 1
        groups.append((b0, nb, int(cbA[b0]), int(cbA[b0 + nb] - cbA[b0]),
                       int(cbB[b0]), int(cbB[b0 + nb] - cbB[b0])))
        b0 += nb

    # per-(window, group) trailing dummy run -> -1 (skipped by the gather via
    # num_idxs_reg); only the run at the END of a group's chunk range can be
    # trimmed since -1s must be trailing within each dma_gather call.
    trimA = []
    trimB = []
    for (b0, nbl, aoff, acnt, boff, bcnt) in groups:
        for idx_arr, off, cnt, trim in ((idxA, aoff, acnt, trimA),
                                        (idxB, boff, bcnt, trimB)):
            if cnt == 0:
                trim.append(0)
                continue
            s0, s1 = off * 128, (off + cnt) * 128
            occ = (idx_arr[:, s0:s1] != 127).any(0)  # real on ANY core
            last = int(np.flatnonzero(occ).max()) if occ.any() else 0
            lastcall = ((cnt - 1) // GCALL) * GCALL * 128  # last call's base
            L = max(last + 1, lastcall + min(256, s1 - s0 - lastcall))
            trim.append(L)
            if s0 + L < s1:
                idx_arr[:, s0 + L : s1] = -1

    return dict(
        sig=sig, pos=pos, halfA=halfA, KA=KA, KB=KB, cbA=cbA, cbB=cbB,
        CA=CA, CB=CB, idxA=wrap(idxA), idxB=wrap(idxB), groups=groups,
        idxA_flat=idxA, idxB_flat=idxB, trimA=trimA, trimB=trimB,
    )


def _prep_weights(theta_w, theta_b, phi_w, phi_b):
    theta_w = np.asarray(theta_w, np.float32)
    phi_w = np.asarray(phi_w, np.float32)
    cb = (np.asarray(theta_b, np.float32) + np.asarray(phi_b, np.float32))
    wcat = np.concatenate(
        [theta_w.transpose(0, 2, 1), (phi_w - theta_w).transpose(0, 2, 1)], axis=2
    )  # [NL, 128(in), 256(out: y|v)]
    return np.ascontiguousarray(wcat.astype(np.float16)), np.ascontiguousarray(cb)


# ----------------------------------------------------------------------------
# device kernel
# ----------------------------------------------------------------------------

def _build_kernel(g):
    import concourse.bacc as bacc
    import concourse.mybir as mybir
    import concourse.tile as tile
    from concourse.masks import make_identity

    KA, KB, groups = g["KA"], g["KB"], g["groups"]
    cbA, cbB = g["cbA"], g["cbB"]
    CA, CB = g["CA"], g["CB"]
    trimA, trimB = g["trimA"], g["trimB"]
    ngroups = len(groups)

    nc = bacc.Bacc("TRN2", target_bir_lowering=False, debug=False,
                   num_devices=NCORES, num_swdge_queues=4)

    xin = nc.dram_tensor("xin", [128, NB * 128], mybir.dt.float16, kind="ExternalInput")
    idxA_in = nc.dram_tensor("idxA", [128, CA * 8], mybir.dt.int16, kind="ExternalInput")
    idxB_in = nc.dram_tensor("idxB", [128, CB * 8], mybir.dt.int16, kind="ExternalInput")
    wcat_in = nc.dram_tensor("wcat", [NL, F, 2 * F], mybir.dt.float16, kind="ExternalInput")
    cb_in = nc.dram_tensor("cb", [NL, F], mybir.dt.float32, kind="ExternalInput")
    xout = nc.dram_tensor("xout", [NPCP, F], mybir.dt.float32, kind="ExternalOutput")

    fp32 = mybir.dt.float32
    fp16 = mybir.dt.float16
    Alu = mybir.AluOpType
    Act = mybir.ActivationFunctionType

    with tile.TileContext(nc) as tc:
        with (
            tc.tile_pool(name="const", bufs=1) as constp,
            tc.tile_pool(name="xp", bufs=2) as xp,
            tc.tile_pool(name="vp", bufs=2) as vp,
            tc.tile_pool(name="wp", bufs=2) as wp,
            tc.tile_pool(name="yp", bufs=3) as yp,
            tc.tile_pool(name="xtp", bufs=3) as xtp,
            tc.tile_pool(name="ga", bufs=LA + 2) as gap,
            tc.tile_pool(name="gb", bufs=5) as gbp,
            tc.tile_pool(name="tp", bufs=4) as tp,
            tc.tile_pool(name="ps", bufs=4, space="PSUM") as ps,
            tc.tile_pool(name="dram", bufs=2, space="DRAM") as dram,
        ):
            ident = constp.tile([128, 128], fp32)
            make_identity(nc, ident[:])
            neg_ph = constp.tile([NPH, F], fp16)
            nc.vector.memset(neg_ph[:], NEG16)
            idxA = constp.tile([128, CA * 8], mybir.dt.int16)
            idxB = constp.tile([128, CB * 8], mybir.dt.int16)
            nc.sync.dma_start(idxA[:], idxA_in[:])
            nc.sync.dma_start(idxB[:], idxB_in[:])

            # layer-0 lhsT comes pre-transposed (and fp16) from the host
            xT0 = constp.tile([128, NB, 128], fp16)
            nc.sync.dma_start(xT0[:], xin.rearrange("p (b l) -> p b l", b=NB))

            # per-layer weight/collective state
            def load_weights(l):
                W = wp.tile([128, 2 * F], fp16, tag="w")
                nc.sync.dma_start(W[:], wcat_in[l])
                cb_sb = wp.tile([1, F], fp32, tag="cb")
                nc.sync.dma_start(cb_sb[:], cb_in[l : l + 1, :])
                cbbc = wp.tile([128, F], fp32, tag="cbbc")
                nc.gpsimd.partition_broadcast(cbbc[:], cb_sb[:])
                return W, cbbc

            def new_tables():
                y1 = dram.tile([H1, F], fp16, tag="y1")
                y2 = dram.tile([H2, F], fp16, tag="y2")
                ya1 = dram.tile([T1, F], fp16, tag="ya1", addr_space="Shared")
                ya2 = dram.tile([T2, F], fp16, tag="ya2", addr_space="Shared")
                return y1, y2, ya1, ya2

            def ag(y, ya):
                nc.gpsimd.collective_compute(
                    "AllGather", Alu.bypass,
                    replica_groups=[list(range(NCORES))],
                    ins=[y.opt()], outs=[ya.opt()],
                )

            def mm_block(b, lhsT, W, cbbc, v, y1, y2):
                yv_ps = ps.tile([128, 2 * F], fp32, tag="yv_ps")
                nc.tensor.matmul(yv_ps[:], lhsT=lhsT, rhs=W[:],
                                 start=True, stop=True)
                y_sb = yp.tile([128, F], fp16, tag="y")
                nc.scalar.activation(y_sb[:], yv_ps[:, 0:F], Act.Copy)
                yt = y1 if b < NB1 else y2
                r0 = (b if b < NB1 else b - NB1) * 128
                if b == 0 or b == NB1:
                    # lanes 117-127 are phantom rows: engines can't address a
                    # partition slice at 117, so write them from a const tile
                    nc.sync.dma_start(yt[r0 : r0 + 128 - NPH, :],
                                      y_sb[0 : 128 - NPH, :])
                    nc.sync.dma_start(yt[r0 + 128 - NPH : r0 + 128, :], neg_ph[:])
                else:
                    nc.sync.dma_start(yt[r0 : r0 + 128, :], y_sb[:])
                nc.vector.tensor_tensor(out=v[:, b, :], in0=yv_ps[:, F : 2 * F],
                                        in1=cbbc[:], op=Alu.add)

            def win_memset(g_tile, cnt, L):
                # skipped trailing lanes of the group's last chunk read as -inf
                if 0 < L < cnt * 128:
                    nc.vector.memset(g_tile[:, (L // 128) : cnt, :], NEG16)

            qn = [0]

            def win_calls(g_tile, ya, idx_sb, off, cnt, L):
                for o in range(0, cnt, GCALL):
                    n = min(GCALL, cnt - o)
                    reg = min(n * 128, L - o * 128)
                    qn[0] = (qn[0] + 1) % 4
                    nc.gpsimd.dma_gather(
                        g_tile[:, o : o + n, :], ya[:, :],
                        idx_sb[:, (off + o) * 8 : (off + o + n) * 8],
                        n * 128, reg, F,
                        queue_num=qn[0],
                    )

            def a_calls(gi, ya1):
                (b0, nbl, aoff, acnt, boff, bcnt) = groups[gi]
                gA = gap.tile([128, GMAX, F], fp16, tag="ga")
                win_memset(gA, acnt, int(trimA[gi]))
                win_calls(gA, ya1, idxA, aoff, acnt, int(trimA[gi]))
                return gA

            def b_tile(gi):
                # allocate + memset one group ahead so the DVE memset never
                # stalls the Pool engine's gather stream
                gB = gbp.tile([128, GMAX, F], fp16, tag="gb")
                win_memset(gB, groups[gi][5], int(trimB[gi]))
                return gB

            # ---- layer 0 matmuls (lhsT pre-transposed on host) ----
            W, cbbc = load_weights(0)
            v = vp.tile([128, NB, F], fp32, tag="v")
            y1, y2, ya1, ya2 = new_tables()
            for b in range(NB):
                mm_block(b, xT0[:, b, :], W, cbbc, v, y1, y2)
                if b == NB1 - 1:
                    ag(y1, ya1)

            for l in range(NL):
                last = l == NL - 1
                if not last:
                    Wn, cbbcn = load_weights(l + 1)
                    vn = vp.tile([128, NB, F], fp32, tag="v")
                    y1n, y2n, ya1n, ya2n = new_tables()
                x_next = xp.tile([128, NB, F], fp32, tag="x")

                # AG2 sits on the Pool queue and holds its SEQ while waiting
                # for the y2 writes; emitting it after one A-call group lets
                # those gathers (which only need AG1) start first, without
                # delaying the AG2 trigger behind too much descriptor-gen.
                ga_tiles = {0: a_calls(0, ya1)}
                ag(y2, ya2)
                for gi in range(1, min(LA, ngroups)):
                    ga_tiles[gi] = a_calls(gi, ya1)

                gb_tiles = {0: b_tile(0)}
                pend = []  # software-pipelined (b, xT) awaiting matmul

                def flush_mm(limit):
                    while len(pend) > limit:
                        bb, xt = pend.pop(0)
                        mm_block(bb, xt[:], Wn, cbbcn, vn, y1n, y2n)
                        if bb == NB1 - 1:
                            ag(y1n, ya1n)

                for gi, (b0, nbl, aoff, acnt, boff, bcnt) in enumerate(groups):
                    gB = gb_tiles.pop(gi)
                    win_calls(gB, ya2, idxB, boff, bcnt, int(trimB[gi]))
                    if gi + 1 < ngroups:
                        gb_tiles[gi + 1] = b_tile(gi + 1)
                    if gi + LA < ngroups:
                        ga_tiles[gi + LA] = a_calls(gi + LA, ya1)
                    gA = ga_tiles.pop(gi)

                    ka = cbA[b0] - aoff
                    kb = cbB[b0] - boff
                    for b in range(b0, b0 + nbl):
                        ha, hb = int(KA[b]), int(KB[b])
                        tS = tp.tile([128, F], fp32, tag="ts")
                        if ha > 0 and hb > 0:
                            tA = tp.tile([128, F], fp32, tag="ta")
                            tB = tp.tile([128, F], fp32, tag="tb")
                            nc.vector.tensor_reduce(
                                out=tA[:],
                                in_=gA[:, ka : ka + ha, :].rearrange("p c f -> p f c"),
                                axis=mybir.AxisListType.X, op=Alu.max)
                            nc.vector.tensor_reduce(
                                out=tB[:],
                                in_=gB[:, kb : kb + hb, :].rearrange("p c f -> p f c"),
                                axis=mybir.AxisListType.X, op=Alu.max)
                            tM = tp.tile([128, F], fp32, tag="tm")
                            nc.vector.tensor_tensor(out=tM[:], in0=tA[:], in1=tB[:],
                                                    op=Alu.max)
                            nc.vector.tensor_tensor(out=tS[:], in0=tM[:],
                                                    in1=v[:, b, :], op=Alu.add)
                        elif ha > 0 or hb > 0:
                            tA = tp.tile([128, F], fp32, tag="ta")
                            src_g = (gA, ka, ha) if ha > 0 else (gB, kb, hb)
                            nc.vector.tensor_reduce(
                                out=tA[:],
                                in_=src_g[0][:, src_g[1] : src_g[1] + src_g[2], :]
                                    .rearrange("p c f -> p f c"),
                                axis=mybir.AxisListType.X, op=Alu.max)
                            nc.vector.tensor_tensor(out=tS[:], in0=tA[:],
                                                    in1=v[:, b, :], op=Alu.add)
                        else:
                            nc.vector.memset(tS[:], NEG)
                        nc.scalar.activation(x_next[:, b, :], tS[:], Act.Relu)
                        if not last:
                            # transpose now, matmul one block behind - keeps
                            # the PE queue from stalling on the ACT xT copy
                            xT_ps = ps.tile([128, 128], fp32, tag="xt_ps")
                            nc.tensor.transpose(xT_ps[:], x_next[:, b, :], ident[:])
                            xT = xtp.tile([128, 128], fp16, tag="xt")
                            nc.scalar.activation(xT[:], xT_ps[:], Act.Copy)
                            pend.append((b, xT))
                            flush_mm(1)
                        ka += ha
                        kb += hb
                if not last:
                    flush_mm(0)
                    W, cbbc, v = Wn, cbbcn, vn
                    y1, y2, ya1, ya2 = y1n, y2n, ya1n, ya2n
                x = x_next

            nc.sync.dma_start(xout.rearrange("(b p) f -> p b f", p=128), x[:])

    nc.compile()
    return nc


# ----------------------------------------------------------------------------
# numpy emulation of the device dataflow (for validating prep structures)
# ----------------------------------------------------------------------------

def _emulate(g, feats_dev, wcat, cb):
    KA, KB = g["KA"], g["KB"]
    x = feats_dev.copy()  # [NCORES, NPCP, F] pos-ordered
    for l in range(NL):
        w = wcat[l].astype(np.float32)
        x16 = x.astype(np.float16).astype(np.float32)
        y_sh = np.einsum("cnf,fk->cnk", x16, w[:, :F]).astype(np.float16)
        v = np.einsum("cnf,fk->cnk", x16, w[:, F:]) + cb[l]
        y_sh[:, 128 - NPH : 128, :] = NEG16
        y_sh[:, H1 + 128 - NPH : H1 + 128, :] = NEG16
        t1 = np.ascontiguousarray(y_sh[:, :H1, :]).reshape(T1, F).astype(np.float32)
        t2 = np.ascontiguousarray(y_sh[:, H1:, :]).reshape(T2, F).astype(np.float32)
        xn = np.empty_like(x)
        for c in range(NCORES):
            iA = g["idxA_flat"][c].astype(np.int64)
            iB = g["idxB_flat"][c].astype(np.int64)
            gA = np.where((iA >= 0)[:, None], t1[np.maximum(iA, 0)], NEG16)
            gB = np.where((iB >= 0)[:, None], t2[np.maximum(iB, 0)], NEG16)
            gA = gA.reshape(g["CA"], 128, F)
            gB = gB.reshape(g["CB"], 128, F)
            for b in range(NB):
                a0, b0 = g["cbA"][b], g["cbB"][b]
                parts = []
                if KA[b] > 0:
                    parts.append(gA[a0 : a0 + KA[b]].max(0))
                if KB[b] > 0:
                    parts.append(gB[b0 : b0 + KB[b]].max(0))
                agg = np.full((128, F), NEG, np.float32) if not parts else (
                    parts[0] if len(parts) == 1 else np.maximum(*parts))
                xn[c, b * 128 : (b + 1) * 128] = np.maximum(
                    agg + v[c, b * 128 : (b + 1) * 128], 0.0)
        x = xn
    return x


def _make_in_maps(g, feats_dev, wcat, cb):
    in_maps = []
    for c in range(NCORES):
        in_maps.append({
            "xin": np.ascontiguousarray(feats_dev[c].T.astype(np.float16)),
            "idxA": np.ascontiguousarray(g["idxA"][c]),
            "idxB": np.ascontiguousarray(g["idxB"][c]),
            "wcat": wcat,
            "cb": cb,
        })
    return in_maps


def _feats_dev(g, feats):
    feats = np.asarray(feats, np.float32)
    fd = np.zeros((NCORES, NPCP, F), np.float32)
    core = np.arange(N) // NPC
    fd[core, g["pos"]] = feats
    return fd


def _assemble(g, results):
    out_sh = np.stack([r["xout"] for r in results])  # [NCORES, NPCP, F]
    core = np.arange(N) // NPC
    return np.ascontiguousarray(out_sh[core, g["pos"]])


def run(feats, src, dst, theta_w, theta_b, phi_w, phi_b, trace=False):
    from concourse.bass_utils import run_bass_kernel_spmd

    key = (src.tobytes()[:64], dst.tobytes()[:64], len(src))
    if _cache.get("graph_key") != key:
        _cache.clear()
        _cache["graph"] = _prep_graph(src, dst)
        _cache["graph_key"] = key
    g = _cache["graph"]
    if "nc" not in _cache:
        _cache["nc"] = _build_kernel(g)
    nc = _cache["nc"]

    wcat, cb = _prep_weights(theta_w, theta_b, phi_w, phi_b)
    feats_dev = _feats_dev(g, feats)
    in_maps = _make_in_maps(g, feats_dev, wcat, cb)
    res = run_bass_kernel_spmd(nc, in_maps, core_ids=list(range(NCORES)),
                               trace=trace)
    out = _assemble(g, res.results)
    return out, res


def kernel(feats, src, dst, theta_w, theta_b, phi_w, phi_b):
    out, _ = run(feats, src, dst, theta_w, theta_b, phi_w, phi_b)
    return out
